# revision 7
# baseline (speedup 1.0000x reference)
"""Trainium2 Bass kernel for nn_LGL GNN message passing (N=64, K=32, F=1024).

Data-parallel over nodes: 8 nodes/core on 8 NeuronCores. Layer-1 adjacency
is sign(x ox s + s ox x); the rank-2 products run on the PE from an exact
6-term bf16 hi/lo split of x and s (residual ~1.6e-5, far below the
sign-flip tolerance). The psum->sbuf sign conversion is split by column
half between the Activation engine (Sign, +-1) and DVE (is_gt-0.5, +-0.5);
the half-dependent scale lands on output f-row blocks and is undone by
pre-scaling the W1 blocks c>=4 by 2. s = sum_k neighbor runs on the idle
GPSIMD engine (partition_all_reduce). Cross-core traffic is two tiny
AllGathers of BN partial sums; each core computes layer 2 + head for its
own 8 nodes only and the host concatenates per-core logits.
"""
import numpy as np

N_CORES = 8
NPC = 8          # nodes per core
F = 1024
K = 32
BN_EPS = 1e-5

_CACHE = {}
_DEBUG = False


def _build():
    import concourse.bacc as bacc
    import concourse.mybir as mybir
    import concourse.bass_isa as bass_isa
    import concourse.tile as tile

    dt = mybir.dt.float32
    dtb = mybir.dt.bfloat16
    AX = mybir.AxisListType
    OP = mybir.AluOpType
    AF = mybir.ActivationFunctionType
    RED = bass_isa.ReduceOp

    nc = bacc.Bacc("TRN2", target_bir_lowering=False, debug=False)

    xs = nc.dram_tensor("xs", [NPC, F], dt, kind="ExternalInput")
    nbs = nc.dram_tensor("nbs", [NPC, K, F], dt, kind="ExternalInput")
    idt64 = nc.dram_tensor("idt64", [64, 64], dt, kind="ExternalInput")
    w1mix = nc.dram_tensor("w1mix", [8, 128, 64], dtb, kind="ExternalInput")
    g4 = nc.dram_tensor("g4", [64, 4], dt, kind="ExternalInput")
    bc4 = nc.dram_tensor("bc4", [4, 64], dt, kind="ExternalInput")
    bnw1 = nc.dram_tensor("bnw1", [64, 1], dt, kind="ExternalInput")
    bnb1 = nc.dram_tensor("bnb1", [64, 1], dt, kind="ExternalInput")
    w2t = nc.dram_tensor("w2t", [64, 32], dt, kind="ExternalInput")
    bnw2 = nc.dram_tensor("bnw2", [32, 1], dt, kind="ExternalInput")
    bnb2 = nc.dram_tensor("bnb2", [32, 1], dt, kind="ExternalInput")
    linw = nc.dram_tensor("linw", [33, 10], dt, kind="ExternalInput")
    out_d = nc.dram_tensor("out", [NPC, 10], dt, kind="ExternalOutput")
    gsh1 = nc.dram_tensor("gsh1", [N_CORES, 4, 2], dt, addr_space="Shared")
    gsh2 = nc.dram_tensor("gsh2", [N_CORES, 32, 2], dt, addr_space="Shared")

    with tile.TileContext(nc) as tc:
        with (
            tc.tile_pool(name="wpool", bufs=1) as wp,
            tc.tile_pool(name="upool", bufs=8) as up,
            tc.tile_pool(name="spool", bufs=2) as sp,
            tc.tile_pool(name="bpool", bufs=2) as bp,
            tc.tile_pool(name="uvpool", bufs=8) as uv,
            tc.tile_pool(name="xpool", bufs=2) as xp,
            tc.tile_pool(name="rpool", bufs=18) as rp,
            tc.tile_pool(name="ypool", bufs=2) as yp,
            tc.tile_pool(name="work", bufs=1) as wk,
            tc.tile_pool(name="psum", bufs=1, space="PSUM") as pp,
            tc.tile_pool(name="dram", bufs=1, space="DRAM") as dp,
        ):
            # ---- load weights / constants ----
            idt64_s = wp.tile([64, 64], dt)
            nc.sync.dma_start(out=idt64_s[:], in_=idt64[:])
            w1mix_s = wp.tile([128, 8, 64], dtb)
            nc.sync.dma_start(out=w1mix_s[:], in_=w1mix.ap().rearrange("c p o -> p c o"))
            g4_s = wp.tile([64, 4], dt)
            nc.sync.dma_start(out=g4_s[:], in_=g4[:])
            bc4_s = wp.tile([4, 64], dt)
            nc.sync.dma_start(out=bc4_s[:], in_=bc4[:])
            bnw1_s = wp.tile([64, 1], dt)
            nc.sync.dma_start(out=bnw1_s[:], in_=bnw1[:])
            bnb1_s = wp.tile([64, 1], dt)
            nc.sync.dma_start(out=bnb1_s[:], in_=bnb1[:])
            w2t_s = wp.tile([64, 32], dt)
            nc.sync.dma_start(out=w2t_s[:], in_=w2t[:])
            bnw2_s = wp.tile([32, 1], dt)
            nc.sync.dma_start(out=bnw2_s[:], in_=bnw2[:])
            bnb2_s = wp.tile([32, 1], dt)
            nc.sync.dma_start(out=bnb2_s[:], in_=bnb2[:])
            linw_s = wp.tile([33, 10], dt)
            nc.sync.dma_start(out=linw_s[:], in_=linw[:])

            # Z[o, n, j]: layer-1 raw outputs; cols 0:32 = nb, col 32 = x
            z_t = wk.tile([64, NPC, 33], dt, tag="z")

            # ======== prologue: loads, s-reduction, bf16 hi/lo splits ====
            u33s = []
            for n in range(NPC):
                u33 = up.tile([33, F], dt, tag="u33", name=f"u33_{n}")
                nc.sync.dma_start(out=u33[0:32, :], in_=nbs[n, :, :])
                nc.sync.dma_start(out=u33[32:33, :], in_=xs[n:n + 1, :])
                u33s.append(u33)

            us6s, vs6s = [], []
            for b in range(2):
                xsb8 = bp.tile([8, F], dt, tag="xsb8", name=f"xsb8_{b}")
                nc.sync.dma_start(out=xsb8[0:4, :], in_=xs[4 * b:4 * b + 4, :])
                for k in range(4):
                    n = 4 * b + k
                    s32 = sp.tile([32, F], dt, tag="s32", name=f"s32_{n}")
                    nc.gpsimd.partition_all_reduce(
                        s32[:], u33s[n][0:32, :], 32, RED.add)
                    nc.sync.dma_start(out=xsb8[4 + k:5 + k, :],
                                      in_=s32[0:1, :])
                hi8 = bp.tile([8, F], dtb, tag="hi8", name=f"hi8_{b}")
                nc.scalar.copy(hi8[:], xsb8[:])
                lo8 = bp.tile([8, F], dtb, tag="lo8", name=f"lo8_{b}")
                nc.vector.tensor_tensor(lo8[:], xsb8[:], hi8[:],
                                        op=OP.subtract)
                for k in range(4):
                    n = 4 * b + k
                    us6 = uv.tile([6, F], dtb, tag="us6", name=f"us6_{n}")
                    vs6 = uv.tile([6, F], dtb, tag="vs6", name=f"vs6_{n}")
                    xh, xl = hi8[k:k + 1, :], lo8[k:k + 1, :]
                    sh_, sl = hi8[4 + k:5 + k, :], lo8[4 + k:5 + k, :]
                    for row, src in ((0, xh), (1, xh), (2, xl),
                                     (3, sh_), (4, sh_), (5, sl)):
                        nc.sync.dma_start(out=us6[row:row + 1, :], in_=src)
                    for row, src in ((0, sh_), (1, sl), (2, sh_),
                                     (3, xh), (4, xl), (5, xh)):
                        nc.sync.dma_start(out=vs6[row:row + 1, :], in_=src)
                    us6s.append(us6)
                    vs6s.append(vs6)

            # ======== layer 1, pipelined over nodes ========
            r_all = [None] * NPC
            x_sbs = [None] * NPC
            accs = [None] * NPC

            def emit_front(n):
                # transposes into packed psum, copy to bf16 X^T
                ps_pack = pp.tile([128, 8, 33], dt, tag="pa", bufs=3,
                                  name=f"pack_{n}")
                nc.scalar.memzero(ps_pack[:])
                for j in range(8):
                    nc.tensor.matmul(ps_pack[:, j, :],
                                     u33s[n][:, j * 128:(j + 1) * 128],
                                     idt64_s[:33, :33], is_transpose=True,
                                     start=False, stop=(j == 7),
                                     skip_group_check=True)
                x_sb = xp.tile([128, 8, 33], dtb, tag="x", name=f"xsb_{n}")
                nc.scalar.copy(
                    x_sb[:].rearrange("p a b -> p (a b)"),
                    ps_pack[:].rearrange("p a b -> p (a b)"))
                x_sbs[n] = x_sb
                r_all[n] = [rp.tile([128, F], dtb, tag="r", name=f"r_{n}_{j}")
                            for j in range(8)]

            def emit_fadj(n, j):
                r_j = r_all[n][j]
                for h in range(2):
                    ps_f = pp.tile([128, 512], dt, tag="f", bufs=4,
                                   name=f"f_{n}_{j}_{h}")
                    nc.tensor.matmul(ps_f[:],
                                     us6s[n][:, j * 128:(j + 1) * 128],
                                     vs6s[n][:, h * 512:(h + 1) * 512],
                                     start=True, stop=True)
                    if h == 0:
                        nc.scalar.sign(r_j[:, 0:512], ps_f[:])
                    else:
                        nc.vector.tensor_scalar(
                            r_j[:, 512:1024], ps_f[:],
                            0.0, 0.5, op0=OP.is_gt, op1=OP.subtract)

            def emit_acc_group(n, i):
                for j in range(8):
                    nc.tensor.matmul(accs[n][:, i, :],
                                     r_all[n][j][:, i * 128:(i + 1) * 128],
                                     x_sbs[n][:, j, :],
                                     start=False, stop=(j == 7),
                                     skip_group_check=True)

            def emit_back(n):
                # y copy + W1 contraction + z copy
                y_sb = yp.tile([128, 8, 33], dtb, tag="y", name=f"y_{n}")
                nc.vector.tensor_copy(
                    y_sb[:].rearrange("p a b -> p (a b)"),
                    accs[n][:].rearrange("p a b -> p (a b)"))
                ps_z = pp.tile([64, 33], dt, tag="zz", bufs=1,
                               name=f"pz_{n}")
                for c in range(8):
                    nc.tensor.matmul(ps_z[:], w1mix_s[:, c, :],
                                     y_sb[:, c, :],
                                     start=(c == 0), stop=(c == 7))
                nc.vector.tensor_copy(z_t[:, n, :], ps_z[:])
                r_all[n] = None
                x_sbs[n] = None
                accs[n] = None

            for n in range(NPC + 1):
                if n < NPC:
                    emit_front(n)
                    accs[n] = pp.tile([128, 8, 33], dt, tag="pa", bufs=3,
                                      name=f"acc_{n}")
                    nc.vector.memset(accs[n][:], 0.0)
                    for j in range(8):
                        emit_fadj(n, j)
                        if n >= 1:
                            emit_acc_group(n - 1, j)
                else:
                    for j in range(8):
                        emit_acc_group(n - 1, j)
                if n >= 1:
                    emit_back(n - 1)

            # ======== BN1 for neighbors (per-node stats over k,f) ========
            sq = wk.tile([64, NPC, 33], dt, tag="sq")
            nc.scalar.square(sq[:], z_t[:])
            ps_s = pp.tile([4, NPC, 33], dt, tag="zz", bufs=1)
            nc.tensor.matmul(ps_s[:], g4_s[:],
                             z_t[:].rearrange("p n j -> p (n j)"),
                             start=True, stop=True)
            ps_q = pp.tile([4, NPC, 33], dt, tag="pa", bufs=3)
            nc.tensor.matmul(ps_q[:], g4_s[:],
                             sq[:].rearrange("p n j -> p (n j)"),
                             start=True, stop=True)
            s_nb = wk.tile([4, NPC], dt, tag="snb")
            q_nb = wk.tile([4, NPC], dt, tag="qnb")
            nc.vector.tensor_reduce(s_nb[:], ps_s[:, :, 0:32], axis=AX.X, op=OP.add)
            nc.vector.tensor_reduce(q_nb[:], ps_q[:, :, 0:32], axis=AX.X, op=OP.add)
            m_nb = wk.tile([4, NPC], dt, tag="mnb")
            nc.vector.tensor_scalar_mul(m_nb[:], s_nb[:], 1.0 / 512)
            v_nb = wk.tile([4, NPC], dt, tag="vnb")
            nc.vector.tensor_scalar(v_nb[:], q_nb[:], 1.0 / 512, BN_EPS,
                                    op0=OP.mult, op1=OP.add)
            m2_nb = wk.tile([4, NPC], dt, tag="m2nb")
            nc.vector.tensor_mul(m2_nb[:], m_nb[:], m_nb[:])
            nc.vector.tensor_sub(v_nb[:], v_nb[:], m2_nb[:])
            nc.scalar.sqrt(v_nb[:], v_nb[:])
            is_nb = wk.tile([4, NPC], dt, tag="isnb")
            nc.vector.reciprocal(is_nb[:], v_nb[:])
            mb_in = wk.tile([4, 16], dt, tag="mbin")
            nc.vector.tensor_copy(mb_in[:, 0:NPC], m_nb[:])
            nc.vector.tensor_copy(mb_in[:, NPC:16], is_nb[:])
            ps_mb = pp.tile([64, 16], dt, tag="zz", bufs=1)
            nc.tensor.matmul(ps_mb[:], bc4_s[:], mb_in[:], start=True, stop=True)
            mb = wk.tile([64, 16], dt, tag="mb")
            nc.vector.tensor_copy(mb[:], ps_mb[:])

            nb1 = wk.tile([64, NPC, K], dt, tag="nb1")
            for n in range(NPC):
                nc.vector.tensor_scalar(nb1[:, n, :], z_t[:, n, 0:32],
                                        mb[:, n:n + 1], mb[:, NPC + n:NPC + n + 1],
                                        op0=OP.subtract, op1=OP.mult)
            nc.vector.tensor_scalar(nb1[:], nb1[:], bnw1_s[:], bnb1_s[:],
                                    op0=OP.mult, op1=OP.add)
            ab1 = wk.tile([64, NPC, K], dt, tag="ab1")
            nc.scalar.activation(ab1[:], nb1[:], AF.Abs)
            nc.gpsimd.tensor_scalar_add(ab1[:], ab1[:], 1.0)
            nc.vector.reciprocal(ab1[:], ab1[:])
            nc.vector.tensor_mul(nb1[:], nb1[:], ab1[:])
            s2_loc = wk.tile([64, NPC], dt, tag="s2loc")
            nc.vector.tensor_reduce(s2_loc[:], nb1[:], axis=AX.X, op=OP.add)

            # ======== BN1-x: allgather partial sums over cores ========
            zx = wk.tile([64, NPC], dt, tag="zx")
            nc.vector.tensor_copy(zx[:], z_t[:, :, 32])
            sqx = wk.tile([64, NPC], dt, tag="sqx")
            nc.scalar.square(sqx[:], zx[:])
            ps_sx = pp.tile([4, NPC], dt, tag="zz", bufs=1)
            nc.tensor.matmul(ps_sx[:], g4_s[:], zx[:], start=True, stop=True)
            ps_qx = pp.tile([4, NPC], dt, tag="pa", bufs=3)
            nc.tensor.matmul(ps_qx[:], g4_s[:], sqx[:], start=True, stop=True)
            gl1 = wk.tile([4, 2], dt, tag="gl1")
            nc.vector.tensor_reduce(gl1[:, 0:1], ps_sx[:], axis=AX.X, op=OP.add)
            nc.vector.tensor_reduce(gl1[:, 1:2], ps_qx[:], axis=AX.X, op=OP.add)
            gb1 = dp.tile([4, 2], dt)
            nc.sync.dma_start(out=gb1[:], in_=gl1[:])
            nc.gpsimd.collective_compute(
                "AllGather", OP.bypass,
                ins=[gb1[:].opt()],
                outs=[gsh1[:].opt()],
                replica_groups=[list(range(N_CORES))],
            )
            gx1 = wk.tile([4, 2, N_CORES], dt, tag="gx1")
            nc.sync.dma_start(
                out=gx1[:], in_=gsh1.ap().rearrange("r c k -> c k r"))
            sx4 = wk.tile([4, 2], dt, tag="sx4")
            nc.vector.tensor_reduce(sx4[:], gx1[:], axis=AX.X, op=OP.add)
            m_x = wk.tile([4, 1], dt, tag="mx")
            nc.vector.tensor_scalar_mul(m_x[:], sx4[:, 0:1], 1.0 / 1024)
            v_x = wk.tile([4, 1], dt, tag="vx")
            nc.vector.tensor_scalar(v_x[:], sx4[:, 1:2], 1.0 / 1024, BN_EPS,
                                    op0=OP.mult, op1=OP.add)
            m2_x = wk.tile([4, 1], dt, tag="m2x")
            nc.vector.tensor_mul(m2_x[:], m_x[:], m_x[:])
            nc.vector.tensor_sub(v_x[:], v_x[:], m2_x[:])
            nc.scalar.sqrt(v_x[:], v_x[:])
            is_x = wk.tile([4, 1], dt, tag="isx")
            nc.vector.reciprocal(is_x[:], v_x[:])
            mbx_in = wk.tile([4, 2], dt, tag="mbxin")
            nc.vector.tensor_copy(mbx_in[:, 0:1], m_x[:])
            nc.vector.tensor_copy(mbx_in[:, 1:2], is_x[:])
            ps_mbx = pp.tile([64, 2], dt, tag="zz", bufs=1)
            nc.tensor.matmul(ps_mbx[:], bc4_s[:], mbx_in[:], start=True, stop=True)
            mbx = wk.tile([64, 2], dt, tag="mbx")
            nc.vector.tensor_copy(mbx[:], ps_mbx[:])

            x1bn = wk.tile([64, NPC], dt, tag="x1bn")
            nc.vector.tensor_scalar(x1bn[:], zx[:], mbx[:, 0:1], mbx[:, 1:2],
                                    op0=OP.subtract, op1=OP.mult)
            nc.vector.tensor_scalar(x1bn[:], x1bn[:], bnw1_s[:], bnb1_s[:],
                                    op0=OP.mult, op1=OP.add)
            abx = wk.tile([64, NPC], dt, tag="abx")
            nc.scalar.activation(abx[:], x1bn[:], AF.Abs)
            nc.vector.tensor_scalar_add(abx[:], abx[:], 1.0)
            nc.vector.reciprocal(abx[:], abx[:])
            nc.vector.tensor_mul(x1bn[:], x1bn[:], abx[:])

            # ======== layer 2 (local 8 nodes only) ========
            ps_t1 = pp.tile([NPC, 64], dt, tag="f", bufs=4)
            nc.tensor.transpose(ps_t1[:], x1bn[:], idt64_s[:])
            x1n = wk.tile([NPC, 64], dt, tag="x1n")
            nc.vector.tensor_copy(x1n[:], ps_t1[:])
            ps_t2 = pp.tile([NPC, 64], dt, tag="f", bufs=4)
            nc.tensor.transpose(ps_t2[:], s2_loc[:], idt64_s[:])
            s2n = wk.tile([NPC, 64], dt, tag="s2n")
            nc.vector.tensor_copy(s2n[:], ps_t2[:])

            sh = [NPC, 4, 16, 16]
            x1_ca = x1n[:].rearrange("p (c a) -> p c a", c=4).unsqueeze(3).broadcast_to(sh)
            x1_cb = x1n[:].rearrange("p (c b) -> p c b", c=4).unsqueeze(2).broadcast_to(sh)
            s2_ca = s2n[:].rearrange("p (c a) -> p c a", c=4).unsqueeze(3).broadcast_to(sh)
            s2_cb = s2n[:].rearrange("p (c b) -> p c b", c=4).unsqueeze(2).broadcast_to(sh)

            f1 = wk.tile(sh, dt, tag="f1")
            f2 = wk.tile(sh, dt, tag="f2")
            nc.vector.tensor_mul(f1[:], x1_ca, s2_cb)
            nc.gpsimd.tensor_tensor(f2[:], x1_cb, s2_ca, op=OP.mult)
            nc.vector.tensor_add(f1[:], f1[:], f2[:])
            sg2 = wk.tile(sh, dt, tag="sg2")
            nc.scalar.sign(sg2[:], f1[:])
            a3 = wk.tile(sh, dt, tag="a3")
            nc.scalar.activation(a3[:], f1[:], AF.Abs)
            b8c = wk.tile([NPC, 1], dt, tag="b8c")
            nc.vector.memset(b8c[:], 1e-8)
            nc.scalar.activation(a3[:], a3[:], AF.Sqrt, bias=b8c[:])
            sr = wk.tile(sh, dt, tag="sr")
            nc.vector.tensor_mul(sr[:], sg2[:], a3[:])
            d01 = wk.tile([NPC, 16, 16], dt, tag="d01")
            d23 = wk.tile([NPC, 16, 16], dt, tag="d23")
            nc.vector.tensor_add(d01[:], a3[:, 0], a3[:, 1])
            nc.gpsimd.tensor_tensor(d23[:], a3[:, 2], a3[:, 3], op=OP.add)
            nc.vector.tensor_add(d01[:], d01[:], d23[:])
            nc.vector.tensor_scalar_add(d01[:], d01[:], 1e-7)
            nc.vector.reciprocal(d01[:], d01[:])
            adj2 = wk.tile(sh, dt, tag="adj2")
            rd_b = d01[:].unsqueeze(1).broadcast_to(sh)
            nc.vector.tensor_mul(adj2[:], sr[:], rd_b)
            p2 = wk.tile(sh, dt, tag="p2")
            nc.gpsimd.tensor_tensor(p2[:], adj2[:], x1_cb, op=OP.mult)
            xa2 = wk.tile([NPC, 4, 16], dt, tag="xa2")
            nc.vector.tensor_reduce(xa2[:], p2[:], axis=AX.X, op=OP.add)
            ps_t3 = pp.tile([64, NPC], dt, tag="f", bufs=4)
            nc.tensor.transpose(ps_t3[:], xa2[:].rearrange("p c a -> p (c a)"),
                                idt64_s[:NPC, :NPC])
            xa2t = wk.tile([64, NPC], dt, tag="xa2t")
            nc.vector.tensor_copy(xa2t[:], ps_t3[:])

            ps_x2 = pp.tile([32, NPC], dt, tag="zz", bufs=1)
            nc.tensor.matmul(ps_x2[:], w2t_s[:], xa2t[:], start=True, stop=True)
            x2 = wk.tile([32, NPC], dt, tag="x2")
            nc.vector.tensor_copy(x2[:], ps_x2[:])

            # ======== BN2: allgather partial sums, then softsign ========
            sq2 = wk.tile([32, NPC], dt, tag="sq2")
            nc.scalar.square(sq2[:], x2[:])
            gl2 = wk.tile([32, 2], dt, tag="gl2")
            nc.vector.tensor_reduce(gl2[:, 0:1], x2[:], axis=AX.X, op=OP.add)
            nc.vector.tensor_reduce(gl2[:, 1:2], sq2[:], axis=AX.X, op=OP.add)
            gb2 = dp.tile([32, 2], dt)
            nc.sync.dma_start(out=gb2[:], in_=gl2[:])
            nc.gpsimd.collective_compute(
                "AllGather", OP.bypass,
                ins=[gb2[:].opt()],
                outs=[gsh2[:].opt()],
                replica_groups=[list(range(N_CORES))],
            )
            gx2 = wk.tile([32, 2, N_CORES], dt, tag="gx2")
            nc.sync.dma_start(
                out=gx2[:], in_=gsh2.ap().rearrange("r c k -> c k r"))
            sx32 = wk.tile([32, 2], dt, tag="sx32")
            nc.vector.tensor_reduce(sx32[:], gx2[:], axis=AX.X, op=OP.add)
            m_2 = wk.tile([32, 1], dt, tag="m2s")
            nc.vector.tensor_scalar_mul(m_2[:], sx32[:, 0:1], 1.0 / 64)
            v_2 = wk.tile([32, 1], dt, tag="v2s")
            nc.vector.tensor_scalar(v_2[:], sx32[:, 1:2], 1.0 / 64, BN_EPS,
                                    op0=OP.mult, op1=OP.add)
            m22 = wk.tile([32, 1], dt, tag="m22s")
            nc.vector.tensor_mul(m22[:], m_2[:], m_2[:])
            nc.vector.tensor_sub(v_2[:], v_2[:], m22[:])
            nc.scalar.sqrt(v_2[:], v_2[:])
            is_2 = wk.tile([32, 1], dt, tag="is2s")
            nc.vector.reciprocal(is_2[:], v_2[:])
            nc.vector.tensor_scalar(x2[:], x2[:], m_2[:], is_2[:],
                                    op0=OP.subtract, op1=OP.mult)
            nc.vector.tensor_scalar(x2[:], x2[:], bnw2_s[:], bnb2_s[:],
                                    op0=OP.mult, op1=OP.add)
            ab2 = wk.tile([32, NPC], dt, tag="ab2")
            nc.scalar.activation(ab2[:], x2[:], AF.Abs)
            nc.vector.tensor_scalar_add(ab2[:], ab2[:], 1.0)
            nc.vector.reciprocal(ab2[:], ab2[:])
            nc.vector.tensor_mul(x2[:], x2[:], ab2[:])

            # linear head: [X2bn; ones]^T @ [lin_w.T; lin_b]
            l33 = wk.tile([33, NPC], dt, tag="l33")
            nc.vector.tensor_copy(l33[0:32, :], x2[:])
            nc.vector.memset(l33[32:33, :], 1.0)
            ps_o = pp.tile([NPC, 10], dt, tag="zz", bufs=1)
            nc.tensor.matmul(ps_o[:], l33[:], linw_s[:], start=True, stop=True)
            o_t = wk.tile([NPC, 10], dt, tag="ot")
            nc.vector.tensor_copy(o_t[:], ps_o[:])
            nc.sync.dma_start(out=out_d[:], in_=o_t[:])

            if _DEBUG:
                for nm, tl in [("dbg_z", z_t), ("dbg_nb1", nb1),
                               ("dbg_s2loc", s2_loc), ("dbg_x1bn", x1bn),
                               ("dbg_xa2", xa2), ("dbg_x2", x2),
                               ("dbg_gx1", gx1)]:
                    d = nc.dram_tensor(nm, list(tl.shape), dt,
                                       kind="ExternalOutput")
                    nc.sync.dma_start(out=d[:], in_=tl[:])

    nc.compile()
    return nc


def _in_maps(x, neighbor, W1, W2, bn1_w, bn1_b, bn2_w, bn2_b, lin_w, lin_b):
    f32 = np.float32
    import ml_dtypes
    bf16 = ml_dtypes.bfloat16
    x = np.ascontiguousarray(x, f32).reshape(64, F)
    nb = np.ascontiguousarray(neighbor, f32).reshape(64, K, F)
    w1f = np.ascontiguousarray(W1, f32).reshape(64, F)
    w1t = np.ascontiguousarray(w1f.T.reshape(8, 128, 64))
    scale = np.ones((8, 1, 1), f32)
    scale[4:] = 2.0     # undo the +-0.5 scale of the DVE-signed column half
    w1mix = (w1t * scale).astype(bf16)
    idt64 = np.eye(64, dtype=f32)
    g4 = np.zeros((64, 4), f32)
    for c in range(4):
        g4[c * 16:(c + 1) * 16, c] = 1.0
    bc4 = np.ascontiguousarray(g4.T)
    bnw1v = np.repeat(np.asarray(bn1_w, f32), 16).reshape(64, 1)
    bnb1v = np.repeat(np.asarray(bn1_b, f32), 16).reshape(64, 1)
    w2t = np.ascontiguousarray(np.asarray(W2, f32).reshape(32, 64).T)
    bnw2v = np.asarray(bn2_w, f32).reshape(32, 1)
    bnb2v = np.asarray(bn2_b, f32).reshape(32, 1)
    linw = np.concatenate([np.asarray(lin_w, f32).T,
                           np.asarray(lin_b, f32).reshape(1, 10)], axis=0)
    maps = []
    for r in range(N_CORES):
        maps.append({
            "xs": np.ascontiguousarray(x[r * NPC:(r + 1) * NPC]),
            "nbs": np.ascontiguousarray(nb[r * NPC:(r + 1) * NPC]),
            "idt64": idt64, "w1mix": w1mix,
            "g4": g4, "bc4": bc4,
            "bnw1": bnw1v, "bnb1": bnb1v, "w2t": w2t,
            "bnw2": bnw2v, "bnb2": bnb2v, "linw": linw,
        })
    return maps


def kernel(**inputs) -> np.ndarray:
    from concourse.bass_utils import run_bass_kernel_spmd
    if "nc" not in _CACHE:
        _CACHE["nc"] = _build()
    nc = _CACHE["nc"]
    maps = _in_maps(**inputs)
    res = run_bass_kernel_spmd(nc, maps, list(range(N_CORES)))
    out = np.concatenate(
        [np.asarray(res.results[r]["out"]) for r in range(N_CORES)], axis=0)
    return np.ascontiguousarray(out, np.float32)
